# revision 52
# baseline (speedup 1.0000x reference)
"""LongRangeProj Bass kernel for TRN2 (8 NeuronCores, channel-sharded).

Math: out[b,c,h,w] = max_{o=(i,j)} x[b,c,o] * exp(-(inv2rv*(fn-|rm|)^2
                                                   + inv2av*wrap(theta-a)^2))
with fn/theta = polar coords of pixel (h,w) around origin o, and the angle
term forced to 1 at the origin pixel itself.  exp is monotone, so the
reduction happens on the exponent: sl = sqa + rdn - ln x, reduced with MIN
over origins, then out = exp(-min).

Key structure (per [128x2048] chunk; partitions = 2 batches x 64 origins):
  t1  = (v + a2) + 1536          # DVE fp16 ts2; fp16 ulp in [1024,2048) is
                                 # 1.0, so this rounds u to the nearest int
                                 # (host pre-wraps a2 to [-0.5, 0.5])
  rr  = t1 - 1536                # DVE ts1 or ACT Identity (load-balanced);
                                 # exact small ints
  rr += -v                       # gpsimd-issued fp16 DMA accumulate (SDMA
                                 # inline ALU) -> rr = round(u) - v.
                                 # NEVER a GPSIMD tensor op: those starve
                                 # DVE ~4x while running.
  sqa = (s2*rr - s2*a2)^2        # ACT Square -> (s2*wrap)^2, bf16
  rdn = (sr*fn + br)^2           # ACT Square, bf16
  sl  = sqa + rdn                # DVE tensor_tensor bf16 (2x mode)
  PSUM: rank-1 matmul seeds every pixel row with -lx[origin] (K=1,
  ones x (-lx) outer product), then 4 transpose-matmuls (sl.T @ I, bf16
  in / fp32 PSUM accumulate) land on top; DVE MIN-reduces origins.
The origin-pixel mask is applied AFTER the reduce: the correct value at
origin o's own pixel is br^2 - lx, precomputed host-side in a per-channel
FIX table (+1e30 elsewhere), folded in with one tiny min; then ACT Exp.
4-deep software pipeline (a1 | dma-wait | a2 | b); first/last channels use
small chunks to shorten fill/drain.  Engine busy ~78us each on ACT & DVE.
"""

import numpy as np
from contextlib import ExitStack

B, C, NH, NW, H, W = 2, 64, 8, 8, 64, 64
STRIDE = 8
NCORES = 8
CL = C // NCORES          # channels per core
HW = H * W                # 4096
NO = NH * NW              # 64 origins
FREE_CHUNK = 2048
NBLK = HW // 128          # 32 pixel blocks of 128
CBIG = float(1.5 * 2 ** 23)   # fp32 round-to-nearest magic constant
TWO_PI = 2.0 * np.pi
NCHUNK = HW // FREE_CHUNK     # 2
BPC = FREE_CHUNK // 128       # 16 blocks per chunk
GPC = BPC // 4                # 4 psum groups per chunk

_built = {}


def _host_fields():
    """Constant geometric fields in [NO, HW] layout, fp32."""
    oy = np.arange(NH, dtype=np.float64) * STRIDE
    ox = np.arange(NW, dtype=np.float64) * STRIDE
    yg = np.arange(H, dtype=np.float64)
    xg = np.arange(W, dtype=np.float64)
    fy = yg[None, :] - oy[:, None]                      # [NH, H]
    fx = xg[None, :] - ox[:, None]                      # [NW, W]
    FY = np.broadcast_to(fy[:, None, :, None], (NH, NW, H, W))
    FX = np.broadcast_to(fx[None, :, None, :], (NH, NW, H, W))
    fn = np.sqrt(FX * FX + FY * FY)
    theta = np.arctan2(FY, FX)
    v = theta / TWO_PI
    rs = lambda a: np.ascontiguousarray(a.reshape(NO, HW).astype(np.float32))
    return rs(v), rs(fn)


def _build_bass():
    import concourse.bass as bass
    import concourse.bacc as bacc
    import concourse.tile as tile
    import concourse.mybir as mybir

    f32 = mybir.dt.float32
    f16 = mybir.dt.bfloat16
    fh = mybir.dt.float16
    AF = mybir.ActivationFunctionType
    OP = mybir.AluOpType
    AX = mybir.AxisListType

    CW = 3 * HW + 128 + 6 * CL + 1 + 64 * CL   # packed const width
    nc = bacc.Bacc("TRN2", target_bir_lowering=False)
    cst_d = nc.dram_tensor("cst", [128, CW], f32, kind="ExternalInput")
    vh_d = nc.dram_tensor("vh", [128, 2 * HW], fh, kind="ExternalInput")
    s16_d = nc.dram_tensor("s16", [128, CL + 1], fh, kind="ExternalInput")
    lxr_d = nc.dram_tensor("lxr", [1, 128 + 512 * CL], f16,
                           kind="ExternalInput")
    # SBUF-natural layout: [channel, partition(pixel-in-block), blk, batch];
    # host reorders to [B, CL, H, W].  Contiguous 256B per partition per DMA.
    out_d = nc.dram_tensor("out", [CL, 128, NBLK, 2], f32,
                           kind="ExternalOutput")

    with ExitStack() as ctx:
        tc = ctx.enter_context(tile.TileContext(nc))
        cpool = ctx.enter_context(tc.tile_pool(name="const", bufs=1))
        front = ctx.enter_context(tc.tile_pool(name="front", bufs=4))
        back = ctx.enter_context(tc.tile_pool(name="back", bufs=3))
        slp = ctx.enter_context(tc.tile_pool(name="slp", bufs=2))
        gp = ctx.enter_context(tc.tile_pool(name="gp", bufs=2))
        psum = ctx.enter_context(tc.tile_pool(name="psum", bufs=8, space="PSUM"))
        outp = ctx.enter_context(tc.tile_pool(name="outp", bufs=4))

        SW = 6 * CL + 1 + 64 * CL
        # fp16 wrap-path constants: V16 = v, VN16 = -v; fp32 FNT for radius
        V16 = cpool.tile([128, HW], fh, tag="V16")
        VN16 = cpool.tile([128, HW], fh, tag="VN16")
        S16 = cpool.tile([128, CL + 1], fh, tag="S16")
        FNT = cpool.tile([128, HW], f32, tag="FNT")
        IDF = cpool.tile([128, 128], f32, tag="IDF")
        SCAL = cpool.tile([128, SW], f32, tag="SCAL")
        IDH = cpool.tile([128, 128], f16, tag="IDH")
        nc.sync.dma_start(SCAL[:, :], cst_d[:, 3 * HW + 128 :])
        nc.sync.dma_start(S16[:, :], s16_d[:, :])
        nc.sync.dma_start(V16[:, 0:512], vh_d[:, 0:512])
        nc.sync.dma_start(VN16[:, 0:512], vh_d[:, HW : HW + 512])
        nc.sync.dma_start(V16[:, 512:], vh_d[:, 512:HW])
        nc.sync.dma_start(VN16[:, 512:], vh_d[:, HW + 512 : 2 * HW])
        nc.sync.dma_start(FNT[:, 0:FREE_CHUNK], cst_d[:, HW : HW + FREE_CHUNK])
        nc.sync.dma_start(FNT[:, FREE_CHUNK:],
                          cst_d[:, HW + FREE_CHUNK : 2 * HW])
        nc.sync.dma_start(IDF[:, :], cst_d[:, 3 * HW : 3 * HW + 128])
        LXR = cpool.tile([1, 128 + 512 * CL], f16, tag="LXR")
        nc.sync.dma_start(LXR[:, :], lxr_d[:, :])
        ONES1 = LXR[:, 0:128]
        A2 = SCAL[:, 0 * CL : 1 * CL]
        S2 = SCAL[:, 1 * CL : 2 * CL]
        SR = SCAL[:, 2 * CL : 3 * CL]
        BR = SCAL[:, 3 * CL : 4 * CL]
        LX = SCAL[:, 4 * CL : 5 * CL]
        NS2A2 = SCAL[:, 5 * CL : 6 * CL]
        NCB = SCAL[:, 6 * CL : 6 * CL + 1]
        FIX = SCAL[:, 6 * CL + 1 :]

        nc.scalar.activation(IDH[:, :], IDF[:, :], AF.Identity)

        # software pipeline; first/last channels use small chunks so the
        # cross-engine dependency chain fills/drains the pipeline faster.
        steps = []   # (it, start, size)
        for it in range(CL):
            if it == 0:
                sizes = [512, 512, 1024, 2048]
            elif it == CL - 1:
                sizes = [2048, 1024, 512, 512]
            else:
                sizes = [2048, 2048]
            pos = 0
            for sz in sizes:
                steps.append((it, pos, sz))
                pos += sz

        o_ts = {}    # per-channel output accumulators

        def stage_a1(it, start, size, k):
            a2h = A2[:, it : it + 1]
            sl_ = slice(start, start + size)
            # fp16 wrap: t1 = (v + a2) + 1536 rounds u to the nearest int
            # (fp16 ulp is 1.0 in [1024, 2048)); rr = t1 - 1536 in {-1,0,1}
            t1 = front.tile([128, size], fh, tag=f"t1_{size}")
            nc.vector.tensor_scalar(t1[:], V16[:, sl_], a2h, 1536.0,
                                    OP.add, OP.add)
            rr = front.tile([128, size], fh, tag=f"rr_{size}")
            if k < 2 or k >= len(steps) - 2:
                # pipeline ramp/drain: compute wu = (v + a2) - rr directly
                # on DVE so these chunks skip the DMA round-trip latency
                nc.vector.tensor_scalar(rr[:], t1[:], 1536.0, None,
                                        OP.subtract)
                wu = front.tile([128, size], fh, tag=f"wu_{size}")
                nc.vector.scalar_tensor_tensor(wu[:], V16[:, sl_], a2h,
                                               rr[:], OP.add, OP.subtract)
                return (wu, True)
            if k % 4 != 0:
                # keep DVE/ACT balanced: 3 of 4 rr ops run on DVE
                nc.vector.tensor_scalar(rr[:], t1[:], 1536.0, None,
                                        OP.subtract)
            else:
                nc.scalar.activation(rr[:], t1[:], AF.Identity, bias=NCB)
            # fp16 DMA accumulate: rr += -v
            nc.gpsimd.dma_start(rr[:], VN16[:, sl_], accum_op=OP.add)
            return (rr, False)

        def stage_a2(it, start, size, rrw):
            rr, is_wu = rrw
            sr = SR[:, it : it + 1]
            br = BR[:, it : it + 1]
            s2 = S2[:, it : it + 1]
            ns2a2 = NS2A2[:, it : it + 1]
            sl_ = slice(start, start + size)
            # ACT: rdn (consts only), bf16
            rdn = back.tile([128, size], f16, tag=f"rdn_{size}")
            nc.scalar.activation(rdn[:], FNT[:, sl_], AF.Square,
                                 scale=sr, bias=br)
            # ACT: sqa = (s2*wrap)^2, bf16.  rr holds either wu (ramp path)
            # or round(u) - v (DMA path, needs the -s2*a2 bias).
            sqa = back.tile([128, size], f16, tag=f"sqa_{size}")
            if is_wu:
                nc.scalar.activation(sqa[:], rr[:], AF.Square, scale=s2)
            else:
                nc.scalar.activation(sqa[:], rr[:], AF.Square, scale=s2,
                                     bias=ns2a2)
            return sqa, rdn

        def stage_b(it, start, size, sqa, rdn):
            nlx = LXR[:, 128 + it * 512 : 128 + (it + 1) * 512]
            sl = slp.tile([128, size], f16, tag=f"sl_{size}")
            nc.vector.tensor_tensor(sl[:], sqa[:], rdn[:], OP.add)
            o_t = o_ts[it]
            for g in range(size // 512):
                ps = psum.tile([128, 512], f32, tag="ps")
                # seed every pixel row with -lx[origin] (rank-1), then
                # accumulate the transposed sl blocks on top
                nc.tensor.matmul(ps[:, :], ONES1[:, :], nlx,
                                 start=True, stop=False)
                for l in range(4):
                    nc.tensor.matmul(
                        ps[:, l * 128 : (l + 1) * 128],
                        sl[:, (g * 4 + l) * 128 : (g * 4 + l + 1) * 128],
                        IDH[:, :], start=False, stop=True)
                red_in = ps[:, :].rearrange("p (q o) -> p q o", q=8, o=64)
                b0 = (start + g * 512) // 128
                nc.vector.tensor_reduce(
                    o_t[:, b0 : b0 + 4, :].rearrange("p a r -> p (a r)"),
                    red_in, axis=AX.X, op=OP.min
                )

        done_px = {}

        def finish_half(it, half):
            # fix + exp + store one half of a channel as soon as its MINs
            # are done, so the final HBM write latency overlaps compute
            cs = slice(half * 32, (half + 1) * 32)
            fix = FIX[:, it * 64 + half * 32 : it * 64 + (half + 1) * 32]
            o_t = o_ts[it]
            o_f = outp.tile([128, 32], f32, tag="o_f")
            nc.vector.scalar_tensor_tensor(
                o_f[:, :],
                o_t[:, :, :].rearrange("p a r -> p (a r)")[:, cs], 0.0,
                fix, OP.add, OP.min)
            o_e = outp.tile([128, 32], f32, tag="o_e")
            nc.scalar.activation(o_e[:, :], o_f[:, :], AF.Exp, scale=-1.0)
            nc.sync.dma_start(
                out_d[it].rearrange("p a r -> p (a r)")[:, cs], o_e[:, :])
            if half == 1:
                o_ts.pop(it)

        # 4-deep pipeline: a1(k) || wait(k-1) || a2(k-2) || b(k-3)
        p0 = None   # (it, start, size, rr)       DMA-in-flight
        p1 = None   # (it, start, size, rr)       awaiting stage_a2
        p2 = None   # (it, start, size, sqa, rdn) awaiting stage_b

        def advance(nxt):
            nonlocal p0, p1, p2
            nxt2 = None
            if p1 is not None:
                it1, st1, sz1, rrw1 = p1
                sqa1, rdn1 = stage_a2(it1, st1, sz1, rrw1)
                nxt2 = (it1, st1, sz1, sqa1, rdn1)
                p1 = None
            if p2 is not None:
                stage_b(*p2)
                it2 = p2[0]
                before = done_px.get(it2, 0)
                after = before + p2[2]
                done_px[it2] = after
                if before < HW // 2 <= after:
                    finish_half(it2, 0)
                if after == HW:
                    finish_half(it2, 1)
                p2 = None
            p2 = nxt2
            p1 = p0
            p0 = nxt

        for k, (it, start, size) in enumerate(steps):
            if start == 0:
                o_t = outp.tile([128, NBLK, 2], f32, tag="o_t")
                o_ts[it] = o_t
            rrw = stage_a1(it, start, size, k)
            advance((it, start, size, rrw))
        advance(None)
        advance(None)
        advance(None)
    nc.finalize()
    return nc


def _host_scalars(x, radius_mean, angle_mean, radius_std, angle_std):
    """Per-core scalar tables [128, CL] + FIX [128, 64*CL].

    partition = b*64 + o.  FIX[8j, 4i*2 + b, for channel slot] = br^2 - lx
    at origin o=(i,j)'s own pixel (h=8i, w=8j -> block 4i, partition 8j).
    """
    inv2rv = 1.0 / (2.0 * (radius_std.astype(np.float64) ** 2 + 0.01))   # [C]
    inv2av = 1.0 / (2.0 * (angle_std.astype(np.float64) ** 2 + 0.0001))  # [C]
    rm = np.abs(radius_mean.astype(np.float64)).reshape(B, C, NO)
    am = angle_mean.astype(np.float64).reshape(B, C, NO)
    xx = np.maximum(x.astype(np.float64).reshape(B, C, NO), 1e-30)
    per_core = []
    for k in range(NCORES):
        cs = np.arange(k * CL, (k + 1) * CL)
        a2 = np.zeros((128, CL)); s2 = np.zeros((128, CL))
        sr = np.zeros((128, CL)); br = np.zeros((128, CL))
        lxv = np.zeros((128, CL)); s2a2 = np.zeros((128, CL))
        fix = np.full((128, CL, NBLK, 2), 1e30)
        for itc, c in enumerate(cs):
            srt = np.sqrt(inv2rv[c])
            for b in range(B):
                p = slice(b * NO, (b + 1) * NO)
                a2r = -am[b, c] / TWO_PI
                a2w = a2r - np.round(a2r)              # wrap to [-0.5, 0.5]
                a2[p, itc] = a2w
                s2v = TWO_PI * np.sqrt(inv2av[c])
                s2[p, itc] = s2v
                sr[p, itc] = srt
                br[p, itc] = -rm[b, c] * srt
                lxv[p, itc] = np.log(xx[b, c])
                s2a2[p, itc] = -s2v * a2w              # sqa bias (-s2*a2)
                for i in range(NH):
                    for j in range(NW):
                        o = i * NW + j
                        brv = np.float32(-rm[b, c, o] * srt)
                        lv = np.float32(np.log(xx[b, c, o]))
                        fix[8 * j, itc, 4 * i, b] = (
                            np.float32(brv * brv) - lv)
        f = lambda a: np.ascontiguousarray(a.astype(np.float32))
        per_core.append(dict(a2=f(a2), s2=f(s2), sr=f(sr), br=f(br),
                             lx=f(lxv), s2a2=f(s2a2),
                             fix=f(fix.reshape(128, CL * NBLK * 2))))
    return per_core


def kernel(x, radius_mean, angle_mean, radius_std, angle_std, _trace=False,
           _tmpdir=None):
    from concourse.bass_utils import run_bass_kernel_spmd

    if "nc" not in _built:
        _built["nc"] = _build_bass()
        _built["fields"] = _host_fields()
    nc = _built["nc"]
    v, fn = _built["fields"]
    fld = np.concatenate([v, fn, -v], axis=1)          # [64, 3*HW]
    vh1 = np.concatenate([v, -v], axis=1)              # [64, 2*HW]
    vh = np.concatenate([vh1, vh1], axis=0).astype(np.float16)
    fld2 = np.concatenate([fld, fld], axis=0)          # [128, 2*HW]
    ident = np.eye(128, dtype=np.float32)
    sc = _host_scalars(x, radius_mean, angle_mean, radius_std, angle_std)
    in_maps = []
    for k in range(NCORES):
        s = sc[k]
        ncb = np.full((128, 1), -1536.0, dtype=np.float32)
        scal = np.concatenate(
            [s["a2"], s["s2"], s["sr"], s["br"], s["lx"], s["s2a2"],
             ncb, s["fix"]], axis=1)
        cst = np.ascontiguousarray(
            np.concatenate([fld2, ident, scal], axis=1))
        s16 = np.concatenate(
            [s["a2"], np.full((128, 1), -1536.0, dtype=np.float32)],
            axis=1).astype(np.float16)
        import ml_dtypes
        lxr = np.empty((1, 128 + 512 * CL), dtype=np.float32)
        lxr[0, :128] = 1.0
        for itc in range(CL):
            nlx = -s["lx"][:, itc]                 # [128] = per (b,o)
            lxr[0, 128 + itc * 512 : 128 + (itc + 1) * 512] = np.tile(nlx, 4)
        lxr = lxr.astype(ml_dtypes.bfloat16)
        in_maps.append({"cst": cst, "vh": vh,
                        "s16": np.ascontiguousarray(s16),
                        "lxr": np.ascontiguousarray(lxr)})
    res = run_bass_kernel_spmd(nc, in_maps, core_ids=list(range(NCORES)),
                               trace=_trace, tmpdir=_tmpdir)
    if _trace:
        return res
    out = np.empty((B, C, H, W), dtype=np.float32)
    for k in range(NCORES):
        r = res.results[k]["out"]          # [CL, 128, NBLK, 2]
        # value at [it, p, blk, b] is pixel blk*128+p of batch b, channel it
        r = r.transpose(3, 0, 2, 1).reshape(B, CL, H, W)
        out[:, k * CL : (k + 1) * CL] = r
    return out


# revision 53
# speedup vs baseline: 1.1859x; 1.1859x over previous
"""LongRangeProj Bass kernel for TRN2 (8 NeuronCores, channel-sharded).

Math: out[b,c,h,w] = max_{o=(i,j)} x[b,c,o] * exp(-(inv2rv*(fn-|rm|)^2
                                                   + inv2av*wrap(theta-a)^2))
with fn/theta = polar coords of pixel (h,w) around origin o, and the angle
term forced to 1 at the origin pixel itself.  exp is monotone, so the
reduction happens on the exponent: sl = sqa + rdn - ln x, reduced with MIN
over origins, then out = exp(-min).

Key structure (per [128x2048] chunk; partitions = 2 batches x 64 origins):
  t1  = (v + a2) + 1536          # DVE fp16 ts2; fp16 ulp in [1024,2048) is
                                 # 1.0, so this rounds u to the nearest int
                                 # (host pre-wraps a2 to [-0.5, 0.5])
  rr  = t1 - 1536                # DVE ts1 or ACT Identity (load-balanced);
                                 # exact small ints
  rr += -v                       # gpsimd-issued fp16 DMA accumulate (SDMA
                                 # inline ALU) -> rr = round(u) - v.
                                 # NEVER a GPSIMD tensor op: those starve
                                 # DVE ~4x while running.
  sqa = (s2*rr - s2*a2)^2        # ACT Square -> (s2*wrap)^2, bf16
  rdn = (sr*fn + br)^2           # ACT Square, bf16
  sl  = sqa + rdn                # DVE tensor_tensor bf16 (2x mode)
  PSUM: rank-1 matmul seeds every pixel row with -lx[origin] (K=1,
  ones x (-lx) outer product), then 4 transpose-matmuls (sl.T @ I, bf16
  in / fp32 PSUM accumulate) land on top; DVE MIN-reduces origins.
The origin-pixel mask is applied AFTER the reduce: the correct value at
origin o's own pixel is br^2 - lx, precomputed host-side in a per-channel
FIX table (+1e30 elsewhere), folded in with one tiny min; then ACT Exp.
4-deep software pipeline (a1 | dma-wait | a2 | b); first/last channels use
small chunks to shorten fill/drain.  Engine busy ~78us each on ACT & DVE.
"""

import numpy as np
from contextlib import ExitStack

B, C, NH, NW, H, W = 2, 64, 8, 8, 64, 64
STRIDE = 8
NCORES = 8
CL = C // NCORES          # channels per core
HW = H * W                # 4096
NO = NH * NW              # 64 origins
FREE_CHUNK = 2048
NBLK = HW // 128          # 32 pixel blocks of 128
CBIG = float(1.5 * 2 ** 23)   # fp32 round-to-nearest magic constant
TWO_PI = 2.0 * np.pi
NCHUNK = HW // FREE_CHUNK     # 2
BPC = FREE_CHUNK // 128       # 16 blocks per chunk
GPC = BPC // 4                # 4 psum groups per chunk

_built = {}


def _host_fields():
    """Constant geometric fields in [NO, HW] layout, fp32."""
    oy = np.arange(NH, dtype=np.float64) * STRIDE
    ox = np.arange(NW, dtype=np.float64) * STRIDE
    yg = np.arange(H, dtype=np.float64)
    xg = np.arange(W, dtype=np.float64)
    fy = yg[None, :] - oy[:, None]                      # [NH, H]
    fx = xg[None, :] - ox[:, None]                      # [NW, W]
    FY = np.broadcast_to(fy[:, None, :, None], (NH, NW, H, W))
    FX = np.broadcast_to(fx[None, :, None, :], (NH, NW, H, W))
    fn = np.sqrt(FX * FX + FY * FY)
    theta = np.arctan2(FY, FX)
    v = theta / TWO_PI
    rs = lambda a: np.ascontiguousarray(a.reshape(NO, HW).astype(np.float32))
    return rs(v), rs(fn)


def _build_bass():
    import concourse.bass as bass
    import concourse.bacc as bacc
    import concourse.tile as tile
    import concourse.mybir as mybir

    f32 = mybir.dt.float32
    f16 = mybir.dt.bfloat16
    fh = mybir.dt.float16
    AF = mybir.ActivationFunctionType
    OP = mybir.AluOpType
    AX = mybir.AxisListType

    CW = 3 * HW + 128 + 6 * CL + 1 + 64 * CL   # packed const width
    nc = bacc.Bacc("TRN2", target_bir_lowering=False)
    cst_d = nc.dram_tensor("cst", [128, CW], f32, kind="ExternalInput")
    vh_d = nc.dram_tensor("vh", [128, 2 * HW], fh, kind="ExternalInput")
    s16_d = nc.dram_tensor("s16", [128, CL + 1], fh, kind="ExternalInput")
    lxr_d = nc.dram_tensor("lxr", [1, 128 + 512 * CL], f16,
                           kind="ExternalInput")
    # SBUF-natural layout: [channel, partition(pixel-in-block), blk, batch];
    # host reorders to [B, CL, H, W].  Contiguous 256B per partition per DMA.
    out_d = nc.dram_tensor("out", [CL, 128, NBLK, 2], f32,
                           kind="ExternalOutput")

    with ExitStack() as ctx:
        tc = ctx.enter_context(tile.TileContext(nc))
        cpool = ctx.enter_context(tc.tile_pool(name="const", bufs=1))
        front = ctx.enter_context(tc.tile_pool(name="front", bufs=4))
        back = ctx.enter_context(tc.tile_pool(name="back", bufs=3))
        slp = ctx.enter_context(tc.tile_pool(name="slp", bufs=2))
        gp = ctx.enter_context(tc.tile_pool(name="gp", bufs=2))
        psum = ctx.enter_context(tc.tile_pool(name="psum", bufs=8, space="PSUM"))
        outp = ctx.enter_context(tc.tile_pool(name="outp", bufs=4))

        SW = 6 * CL + 1 + 64 * CL
        # fp16 wrap-path constants: V16 = v, VN16 = -v; fp32 FNT for radius
        V16 = cpool.tile([128, HW], fh, tag="V16")
        VN16 = cpool.tile([128, HW], fh, tag="VN16")
        S16 = cpool.tile([128, CL + 1], fh, tag="S16")
        FNT = cpool.tile([128, HW], f32, tag="FNT")
        IDF = cpool.tile([128, 128], f32, tag="IDF")
        SCAL = cpool.tile([128, SW], f32, tag="SCAL")
        IDH = cpool.tile([128, 128], f16, tag="IDH")
        nc.sync.dma_start(SCAL[:, :], cst_d[:, 3 * HW + 128 :])
        nc.sync.dma_start(S16[:, :], s16_d[:, :])
        nc.sync.dma_start(V16[:, 0:512], vh_d[:, 0:512])
        nc.sync.dma_start(VN16[:, 0:512], vh_d[:, HW : HW + 512])
        nc.sync.dma_start(V16[:, 512:], vh_d[:, 512:HW])
        nc.sync.dma_start(VN16[:, 512:], vh_d[:, HW + 512 : 2 * HW])
        nc.sync.dma_start(FNT[:, 0:FREE_CHUNK], cst_d[:, HW : HW + FREE_CHUNK])
        nc.sync.dma_start(FNT[:, FREE_CHUNK:],
                          cst_d[:, HW + FREE_CHUNK : 2 * HW])
        nc.sync.dma_start(IDF[:, :], cst_d[:, 3 * HW : 3 * HW + 128])
        LXR = cpool.tile([1, 128 + 512 * CL], f16, tag="LXR")
        nc.sync.dma_start(LXR[:, :], lxr_d[:, :])
        ONES1 = LXR[:, 0:128]
        A2 = SCAL[:, 0 * CL : 1 * CL]
        S2 = SCAL[:, 1 * CL : 2 * CL]
        SR = SCAL[:, 2 * CL : 3 * CL]
        BR = SCAL[:, 3 * CL : 4 * CL]
        LX = SCAL[:, 4 * CL : 5 * CL]
        NS2A2 = SCAL[:, 5 * CL : 6 * CL]
        NCB = SCAL[:, 6 * CL : 6 * CL + 1]
        FIX = SCAL[:, 6 * CL + 1 :]

        nc.scalar.activation(IDH[:, :], IDF[:, :], AF.Identity)

        # software pipeline; first/last channels use small chunks so the
        # cross-engine dependency chain fills/drains the pipeline faster.
        steps = []   # (it, start, size)
        for it in range(CL):
            if it == 0:
                sizes = [512, 512, 1024, 2048]
            elif it == CL - 1:
                sizes = [2048, 1024, 512, 512]
            else:
                sizes = [2048, 2048]
            pos = 0
            for sz in sizes:
                steps.append((it, pos, sz))
                pos += sz

        o_ts = {}    # per-channel output accumulators

        def stage_a1(it, start, size, k):
            a2h = A2[:, it : it + 1]
            sl_ = slice(start, start + size)
            # fp16 wrap: t1 = (v + a2) + 1536 rounds u to the nearest int
            # (fp16 ulp is 1.0 in [1024, 2048)); rr = t1 - 1536 in {-1,0,1}
            t1 = front.tile([128, size], fh, tag=f"t1_{size}")
            nc.vector.tensor_scalar(t1[:], V16[:, sl_], a2h, 1536.0,
                                    OP.add, OP.add)
            rr = front.tile([128, size], fh, tag=f"rr_{size}")
            if k < 2 or k >= len(steps) - 2:
                # pipeline ramp/drain: compute wu = (v + a2) - rr directly
                # on DVE so these chunks skip the DMA round-trip latency
                nc.vector.tensor_scalar(rr[:], t1[:], 1536.0, None,
                                        OP.subtract)
                wu = front.tile([128, size], fh, tag=f"wu_{size}")
                nc.vector.scalar_tensor_tensor(wu[:], V16[:, sl_], a2h,
                                               rr[:], OP.add, OP.subtract)
                return (wu, True)
            if k % 4 != 0:
                # keep DVE/ACT balanced: 3 of 4 rr ops run on DVE
                nc.vector.tensor_scalar(rr[:], t1[:], 1536.0, None,
                                        OP.subtract)
            else:
                nc.scalar.activation(rr[:], t1[:], AF.Identity, bias=NCB)
            # fp16 DMA accumulate: rr += -v
            nc.gpsimd.dma_start(rr[:], VN16[:, sl_], accum_op=OP.add)
            return (rr, False)

        def stage_a2(it, start, size, rrw):
            rr, is_wu = rrw
            sr = SR[:, it : it + 1]
            br = BR[:, it : it + 1]
            s2 = S2[:, it : it + 1]
            ns2a2 = NS2A2[:, it : it + 1]
            sl_ = slice(start, start + size)
            # ACT: rdn (consts only), bf16
            rdn = back.tile([128, size], f16, tag=f"rdn_{size}")
            nc.scalar.activation(rdn[:], FNT[:, sl_], AF.Square,
                                 scale=sr, bias=br)
            # ACT: sqa = (s2*wrap)^2, bf16.  rr holds either wu (ramp path)
            # or round(u) - v (DMA path, needs the -s2*a2 bias).
            sqa = back.tile([128, size], f16, tag=f"sqa_{size}")
            if is_wu:
                nc.scalar.activation(sqa[:], rr[:], AF.Square, scale=s2)
            else:
                nc.scalar.activation(sqa[:], rr[:], AF.Square, scale=s2,
                                     bias=ns2a2)
            return sqa, rdn

        def stage_b(it, start, size, sqa, rdn):
            nlx = LXR[:, 128 + it * 512 : 128 + (it + 1) * 512]
            sl = slp.tile([128, size], f16, tag=f"sl_{size}")
            nc.vector.tensor_tensor(sl[:], sqa[:], rdn[:], OP.add)
            o_t = o_ts[it]
            for g in range(size // 512):
                ps = psum.tile([128, 512], f32, tag="ps")
                # seed every pixel row with -lx[origin] (rank-1), then
                # accumulate the transposed sl blocks on top
                nc.tensor.matmul(ps[:, :], ONES1[:, :], nlx,
                                 start=True, stop=False)
                for l in range(4):
                    nc.tensor.matmul(
                        ps[:, l * 128 : (l + 1) * 128],
                        sl[:, (g * 4 + l) * 128 : (g * 4 + l + 1) * 128],
                        IDH[:, :], start=False, stop=True)
                red_in = ps[:, :].rearrange("p (q o) -> p q o", q=8, o=64)
                b0 = (start + g * 512) // 128
                nc.vector.tensor_reduce(
                    o_t[:, b0 : b0 + 4, :].rearrange("p a r -> p (a r)"),
                    red_in, axis=AX.X, op=OP.min
                )

        def finish_channel(it):
            fix = FIX[:, it * 64 : (it + 1) * 64]
            o_t = o_ts.pop(it)
            o_f = outp.tile([128, NBLK * 2], f32, tag="o_f")
            nc.vector.scalar_tensor_tensor(
                o_f[:, :], o_t[:, :, :].rearrange("p a r -> p (a r)"), 0.0,
                fix, OP.add, OP.min)
            o_e = outp.tile([128, NBLK * 2], f32, tag="o_e")
            nc.scalar.activation(o_e[:, :], o_f[:, :], AF.Exp, scale=-1.0)
            nc.sync.dma_start(
                out_d[it].rearrange("p a r -> p (a r)"), o_e[:, :])

        # 4-deep pipeline: a1(k) || wait(k-1) || a2(k-2) || b(k-3)
        p0 = None   # (it, start, size, rr)       DMA-in-flight
        p1 = None   # (it, start, size, rr)       awaiting stage_a2
        p2 = None   # (it, start, size, sqa, rdn) awaiting stage_b

        def advance(nxt):
            nonlocal p0, p1, p2
            nxt2 = None
            if p1 is not None:
                it1, st1, sz1, rrw1 = p1
                sqa1, rdn1 = stage_a2(it1, st1, sz1, rrw1)
                nxt2 = (it1, st1, sz1, sqa1, rdn1)
                p1 = None
            if p2 is not None:
                stage_b(*p2)
                if p2[1] + p2[2] == HW:
                    finish_channel(p2[0])
                p2 = None
            p2 = nxt2
            p1 = p0
            p0 = nxt

        for k, (it, start, size) in enumerate(steps):
            if start == 0:
                o_t = outp.tile([128, NBLK, 2], f32, tag="o_t")
                o_ts[it] = o_t
            rrw = stage_a1(it, start, size, k)
            advance((it, start, size, rrw))
        advance(None)
        advance(None)
        advance(None)
    nc.finalize()
    return nc


def _host_scalars(x, radius_mean, angle_mean, radius_std, angle_std):
    """Per-core scalar tables [128, CL] + FIX [128, 64*CL].

    partition = b*64 + o.  FIX[8j, 4i*2 + b, for channel slot] = br^2 - lx
    at origin o=(i,j)'s own pixel (h=8i, w=8j -> block 4i, partition 8j).
    """
    inv2rv = 1.0 / (2.0 * (radius_std.astype(np.float64) ** 2 + 0.01))   # [C]
    inv2av = 1.0 / (2.0 * (angle_std.astype(np.float64) ** 2 + 0.0001))  # [C]
    rm = np.abs(radius_mean.astype(np.float64)).reshape(B, C, NO)
    am = angle_mean.astype(np.float64).reshape(B, C, NO)
    xx = np.maximum(x.astype(np.float64).reshape(B, C, NO), 1e-30)
    per_core = []
    for k in range(NCORES):
        cs = np.arange(k * CL, (k + 1) * CL)
        a2 = np.zeros((128, CL)); s2 = np.zeros((128, CL))
        sr = np.zeros((128, CL)); br = np.zeros((128, CL))
        lxv = np.zeros((128, CL)); s2a2 = np.zeros((128, CL))
        fix = np.full((128, CL, NBLK, 2), 1e30)
        for itc, c in enumerate(cs):
            srt = np.sqrt(inv2rv[c])
            for b in range(B):
                p = slice(b * NO, (b + 1) * NO)
                a2r = -am[b, c] / TWO_PI
                a2w = a2r - np.round(a2r)              # wrap to [-0.5, 0.5]
                a2[p, itc] = a2w
                s2v = TWO_PI * np.sqrt(inv2av[c])
                s2[p, itc] = s2v
                sr[p, itc] = srt
                br[p, itc] = -rm[b, c] * srt
                lxv[p, itc] = np.log(xx[b, c])
                s2a2[p, itc] = -s2v * a2w              # sqa bias (-s2*a2)
                for i in range(NH):
                    for j in range(NW):
                        o = i * NW + j
                        brv = np.float32(-rm[b, c, o] * srt)
                        lv = np.float32(np.log(xx[b, c, o]))
                        fix[8 * j, itc, 4 * i, b] = (
                            np.float32(brv * brv) - lv)
        f = lambda a: np.ascontiguousarray(a.astype(np.float32))
        per_core.append(dict(a2=f(a2), s2=f(s2), sr=f(sr), br=f(br),
                             lx=f(lxv), s2a2=f(s2a2),
                             fix=f(fix.reshape(128, CL * NBLK * 2))))
    return per_core


def kernel(x, radius_mean, angle_mean, radius_std, angle_std, _trace=False,
           _tmpdir=None):
    from concourse.bass_utils import run_bass_kernel_spmd

    if "nc" not in _built:
        _built["nc"] = _build_bass()
        _built["fields"] = _host_fields()
    nc = _built["nc"]
    v, fn = _built["fields"]
    fld = np.concatenate([v, fn, -v], axis=1)          # [64, 3*HW]
    vh1 = np.concatenate([v, -v], axis=1)              # [64, 2*HW]
    vh = np.concatenate([vh1, vh1], axis=0).astype(np.float16)
    fld2 = np.concatenate([fld, fld], axis=0)          # [128, 2*HW]
    ident = np.eye(128, dtype=np.float32)
    sc = _host_scalars(x, radius_mean, angle_mean, radius_std, angle_std)
    in_maps = []
    for k in range(NCORES):
        s = sc[k]
        ncb = np.full((128, 1), -1536.0, dtype=np.float32)
        scal = np.concatenate(
            [s["a2"], s["s2"], s["sr"], s["br"], s["lx"], s["s2a2"],
             ncb, s["fix"]], axis=1)
        cst = np.ascontiguousarray(
            np.concatenate([fld2, ident, scal], axis=1))
        s16 = np.concatenate(
            [s["a2"], np.full((128, 1), -1536.0, dtype=np.float32)],
            axis=1).astype(np.float16)
        import ml_dtypes
        lxr = np.empty((1, 128 + 512 * CL), dtype=np.float32)
        lxr[0, :128] = 1.0
        for itc in range(CL):
            nlx = -s["lx"][:, itc]                 # [128] = per (b,o)
            lxr[0, 128 + itc * 512 : 128 + (itc + 1) * 512] = np.tile(nlx, 4)
        lxr = lxr.astype(ml_dtypes.bfloat16)
        in_maps.append({"cst": cst, "vh": vh,
                        "s16": np.ascontiguousarray(s16),
                        "lxr": np.ascontiguousarray(lxr)})
    res = run_bass_kernel_spmd(nc, in_maps, core_ids=list(range(NCORES)),
                               trace=_trace, tmpdir=_tmpdir)
    if _trace:
        return res
    out = np.empty((B, C, H, W), dtype=np.float32)
    for k in range(NCORES):
        r = res.results[k]["out"]          # [CL, 128, NBLK, 2]
        # value at [it, p, blk, b] is pixel blk*128+p of batch b, channel it
        r = r.transpose(3, 0, 2, 1).reshape(B, CL, H, W)
        out[:, k * CL : (k + 1) * CL] = r
    return out


# revision 54
# speedup vs baseline: 1.1887x; 1.0024x over previous
"""LongRangeProj Bass kernel for TRN2 (8 NeuronCores, channel-sharded).

Math: out[b,c,h,w] = max_{o=(i,j)} x[b,c,o] * exp(-(inv2rv*(fn-|rm|)^2
                                                   + inv2av*wrap(theta-a)^2))
with fn/theta = polar coords of pixel (h,w) around origin o, and the angle
term forced to 1 at the origin pixel itself.  exp is monotone, so the
reduction happens on the exponent: sl = sqa + rdn - ln x, reduced with MIN
over origins, then out = exp(-min).

Key structure (per [128x2048] chunk; partitions = 2 batches x 64 origins):
  t1  = (v + a2) + 1536          # DVE fp16 ts2; fp16 ulp in [1024,2048) is
                                 # 1.0, so this rounds u to the nearest int
                                 # (host pre-wraps a2 to [-0.5, 0.5])
  rr  = t1 - 1536                # DVE ts1 or ACT Identity (load-balanced);
                                 # exact small ints
  rr += -v                       # gpsimd-issued fp16 DMA accumulate (SDMA
                                 # inline ALU) -> rr = round(u) - v.
                                 # NEVER a GPSIMD tensor op: those starve
                                 # DVE ~4x while running.
  sqa = (s2*rr - s2*a2)^2        # ACT Square -> (s2*wrap)^2, bf16
  rdn = (sr*fn + br)^2           # ACT Square, bf16
  sl  = sqa + rdn                # DVE tensor_tensor bf16 (2x mode)
  PSUM: rank-1 matmul seeds every pixel row with -lx[origin] (K=1,
  ones x (-lx) outer product), then 4 transpose-matmuls (sl.T @ I, bf16
  in / fp32 PSUM accumulate) land on top; DVE MIN-reduces origins.
The origin-pixel mask is applied AFTER the reduce: the correct value at
origin o's own pixel is br^2 - lx, precomputed host-side in a per-channel
FIX table (+1e30 elsewhere), folded in with one tiny min; then ACT Exp.
4-deep software pipeline (a1 | dma-wait | a2 | b); first/last channels use
small chunks to shorten fill/drain.  Engine busy ~78us each on ACT & DVE.
"""

import numpy as np
from contextlib import ExitStack

B, C, NH, NW, H, W = 2, 64, 8, 8, 64, 64
STRIDE = 8
NCORES = 8
CL = C // NCORES          # channels per core
HW = H * W                # 4096
NO = NH * NW              # 64 origins
FREE_CHUNK = 2048
NBLK = HW // 128          # 32 pixel blocks of 128
CBIG = float(1.5 * 2 ** 23)   # fp32 round-to-nearest magic constant
TWO_PI = 2.0 * np.pi
NCHUNK = HW // FREE_CHUNK     # 2
BPC = FREE_CHUNK // 128       # 16 blocks per chunk
GPC = BPC // 4                # 4 psum groups per chunk

_built = {}


def _host_fields():
    """Constant geometric fields in [NO, HW] layout, fp32."""
    oy = np.arange(NH, dtype=np.float64) * STRIDE
    ox = np.arange(NW, dtype=np.float64) * STRIDE
    yg = np.arange(H, dtype=np.float64)
    xg = np.arange(W, dtype=np.float64)
    fy = yg[None, :] - oy[:, None]                      # [NH, H]
    fx = xg[None, :] - ox[:, None]                      # [NW, W]
    FY = np.broadcast_to(fy[:, None, :, None], (NH, NW, H, W))
    FX = np.broadcast_to(fx[None, :, None, :], (NH, NW, H, W))
    fn = np.sqrt(FX * FX + FY * FY)
    theta = np.arctan2(FY, FX)
    v = theta / TWO_PI
    rs = lambda a: np.ascontiguousarray(a.reshape(NO, HW).astype(np.float32))
    return rs(v), rs(fn)


def _build_bass():
    import concourse.bass as bass
    import concourse.bacc as bacc
    import concourse.tile as tile
    import concourse.mybir as mybir

    f32 = mybir.dt.float32
    f16 = mybir.dt.bfloat16
    fh = mybir.dt.float16
    AF = mybir.ActivationFunctionType
    OP = mybir.AluOpType
    AX = mybir.AxisListType

    CW = 3 * HW + 128 + 6 * CL + 1 + 64 * CL   # packed const width
    nc = bacc.Bacc("TRN2", target_bir_lowering=False)
    cst_d = nc.dram_tensor("cst", [128, CW], f32, kind="ExternalInput")
    vh_d = nc.dram_tensor("vh", [128, 2 * HW], fh, kind="ExternalInput")
    s16_d = nc.dram_tensor("s16", [128, CL + 1], fh, kind="ExternalInput")
    lxr_d = nc.dram_tensor("lxr", [1, 128 + 512 * CL], f16,
                           kind="ExternalInput")
    # SBUF-natural layout: [channel, partition(pixel-in-block), blk, batch];
    # host reorders to [B, CL, H, W].  Contiguous 256B per partition per DMA.
    out_d = nc.dram_tensor("out", [CL, 128, NBLK, 2], f32,
                           kind="ExternalOutput")

    with ExitStack() as ctx:
        tc = ctx.enter_context(tile.TileContext(nc))
        cpool = ctx.enter_context(tc.tile_pool(name="const", bufs=1))
        front = ctx.enter_context(tc.tile_pool(name="front", bufs=4))
        back = ctx.enter_context(tc.tile_pool(name="back", bufs=3))
        slp = ctx.enter_context(tc.tile_pool(name="slp", bufs=2))
        gp = ctx.enter_context(tc.tile_pool(name="gp", bufs=2))
        psum = ctx.enter_context(tc.tile_pool(name="psum", bufs=8, space="PSUM"))
        outp = ctx.enter_context(tc.tile_pool(name="outp", bufs=4))

        SW = 6 * CL + 1 + 64 * CL
        # fp16 wrap-path constants: V16 = v, VN16 = -v; fp32 FNT for radius
        V16 = cpool.tile([128, HW], fh, tag="V16")
        VN16 = cpool.tile([128, HW], fh, tag="VN16")
        S16 = cpool.tile([128, CL + 1], fh, tag="S16")
        FNT = cpool.tile([128, HW], f32, tag="FNT")
        IDF = cpool.tile([128, 128], f32, tag="IDF")
        SCAL = cpool.tile([128, SW], f32, tag="SCAL")
        IDH = cpool.tile([128, 128], f16, tag="IDH")
        nc.sync.dma_start(SCAL[:, :], cst_d[:, 3 * HW + 128 :])
        nc.sync.dma_start(S16[:, :], s16_d[:, :])
        nc.sync.dma_start(V16[:, 0:512], vh_d[:, 0:512])
        nc.sync.dma_start(VN16[:, 0:512], vh_d[:, HW : HW + 512])
        nc.sync.dma_start(V16[:, 512:], vh_d[:, 512:HW])
        nc.sync.dma_start(VN16[:, 512:], vh_d[:, HW + 512 : 2 * HW])
        nc.sync.dma_start(FNT[:, 0:512], cst_d[:, HW : HW + 512])
        nc.sync.dma_start(FNT[:, 512:FREE_CHUNK],
                          cst_d[:, HW + 512 : HW + FREE_CHUNK])
        nc.sync.dma_start(FNT[:, FREE_CHUNK:],
                          cst_d[:, HW + FREE_CHUNK : 2 * HW])
        nc.sync.dma_start(IDF[:, :], cst_d[:, 3 * HW : 3 * HW + 128])
        LXR = cpool.tile([1, 128 + 512 * CL], f16, tag="LXR")
        nc.sync.dma_start(LXR[:, :], lxr_d[:, :])
        ONES1 = LXR[:, 0:128]
        A2 = SCAL[:, 0 * CL : 1 * CL]
        S2 = SCAL[:, 1 * CL : 2 * CL]
        SR = SCAL[:, 2 * CL : 3 * CL]
        BR = SCAL[:, 3 * CL : 4 * CL]
        LX = SCAL[:, 4 * CL : 5 * CL]
        NS2A2 = SCAL[:, 5 * CL : 6 * CL]
        NCB = SCAL[:, 6 * CL : 6 * CL + 1]
        FIX = SCAL[:, 6 * CL + 1 :]

        nc.scalar.activation(IDH[:, :], IDF[:, :], AF.Identity)

        # software pipeline; first/last channels use small chunks so the
        # cross-engine dependency chain fills/drains the pipeline faster.
        steps = []   # (it, start, size)
        for it in range(CL):
            if it == 0:
                sizes = [512, 512, 1024, 2048]
            elif it == CL - 1:
                sizes = [2048, 1024, 512, 512]
            else:
                sizes = [2048, 2048]
            pos = 0
            for sz in sizes:
                steps.append((it, pos, sz))
                pos += sz

        o_ts = {}    # per-channel output accumulators

        def stage_a1(it, start, size, k):
            a2h = A2[:, it : it + 1]
            sl_ = slice(start, start + size)
            # fp16 wrap: t1 = (v + a2) + 1536 rounds u to the nearest int
            # (fp16 ulp is 1.0 in [1024, 2048)); rr = t1 - 1536 in {-1,0,1}
            t1 = front.tile([128, size], fh, tag=f"t1_{size}")
            nc.vector.tensor_scalar(t1[:], V16[:, sl_], a2h, 1536.0,
                                    OP.add, OP.add)
            rr = front.tile([128, size], fh, tag=f"rr_{size}")
            if k < 2 or k >= len(steps) - 2:
                # pipeline ramp/drain: compute wu = (v + a2) - rr directly
                # on DVE so these chunks skip the DMA round-trip latency
                nc.vector.tensor_scalar(rr[:], t1[:], 1536.0, None,
                                        OP.subtract)
                wu = front.tile([128, size], fh, tag=f"wu_{size}")
                nc.vector.scalar_tensor_tensor(wu[:], V16[:, sl_], a2h,
                                               rr[:], OP.add, OP.subtract)
                return (wu, True)
            if k % 4 != 0:
                # keep DVE/ACT balanced: 3 of 4 rr ops run on DVE
                nc.vector.tensor_scalar(rr[:], t1[:], 1536.0, None,
                                        OP.subtract)
            else:
                nc.scalar.activation(rr[:], t1[:], AF.Identity, bias=NCB)
            # fp16 DMA accumulate: rr += -v
            nc.gpsimd.dma_start(rr[:], VN16[:, sl_], accum_op=OP.add)
            return (rr, False)

        def stage_a2(it, start, size, rrw):
            rr, is_wu = rrw
            sr = SR[:, it : it + 1]
            br = BR[:, it : it + 1]
            s2 = S2[:, it : it + 1]
            ns2a2 = NS2A2[:, it : it + 1]
            sl_ = slice(start, start + size)
            # ACT: rdn (consts only), bf16
            rdn = back.tile([128, size], f16, tag=f"rdn_{size}")
            nc.scalar.activation(rdn[:], FNT[:, sl_], AF.Square,
                                 scale=sr, bias=br)
            # ACT: sqa = (s2*wrap)^2, bf16.  rr holds either wu (ramp path)
            # or round(u) - v (DMA path, needs the -s2*a2 bias).
            sqa = back.tile([128, size], f16, tag=f"sqa_{size}")
            if is_wu:
                nc.scalar.activation(sqa[:], rr[:], AF.Square, scale=s2)
            else:
                nc.scalar.activation(sqa[:], rr[:], AF.Square, scale=s2,
                                     bias=ns2a2)
            return sqa, rdn

        def stage_b(it, start, size, sqa, rdn):
            nlx = LXR[:, 128 + it * 512 : 128 + (it + 1) * 512]
            sl = slp.tile([128, size], f16, tag=f"sl_{size}")
            nc.vector.tensor_tensor(sl[:], sqa[:], rdn[:], OP.add)
            o_t = o_ts[it]
            for g in range(size // 512):
                ps = psum.tile([128, 512], f32, tag="ps")
                # seed every pixel row with -lx[origin] (rank-1), then
                # accumulate the transposed sl blocks on top
                nc.tensor.matmul(ps[:, :], ONES1[:, :], nlx,
                                 start=True, stop=False)
                for l in range(4):
                    nc.tensor.matmul(
                        ps[:, l * 128 : (l + 1) * 128],
                        sl[:, (g * 4 + l) * 128 : (g * 4 + l + 1) * 128],
                        IDH[:, :], start=False, stop=True)
                red_in = ps[:, :].rearrange("p (q o) -> p q o", q=8, o=64)
                b0 = (start + g * 512) // 128
                nc.vector.tensor_reduce(
                    o_t[:, b0 : b0 + 4, :].rearrange("p a r -> p (a r)"),
                    red_in, axis=AX.X, op=OP.min
                )

        def finish_channel(it):
            fix = FIX[:, it * 64 : (it + 1) * 64]
            o_t = o_ts.pop(it)
            o_f = outp.tile([128, NBLK * 2], f32, tag="o_f")
            nc.vector.scalar_tensor_tensor(
                o_f[:, :], o_t[:, :, :].rearrange("p a r -> p (a r)"), 0.0,
                fix, OP.add, OP.min)
            o_e = outp.tile([128, NBLK * 2], f32, tag="o_e")
            nc.scalar.activation(o_e[:, :], o_f[:, :], AF.Exp, scale=-1.0)
            nc.sync.dma_start(
                out_d[it].rearrange("p a r -> p (a r)"), o_e[:, :])

        # 4-deep pipeline: a1(k) || wait(k-1) || a2(k-2) || b(k-3)
        p0 = None   # (it, start, size, rr)       DMA-in-flight
        p1 = None   # (it, start, size, rr)       awaiting stage_a2
        p2 = None   # (it, start, size, sqa, rdn) awaiting stage_b

        def advance(nxt):
            nonlocal p0, p1, p2
            nxt2 = None
            if p1 is not None:
                it1, st1, sz1, rrw1 = p1
                sqa1, rdn1 = stage_a2(it1, st1, sz1, rrw1)
                nxt2 = (it1, st1, sz1, sqa1, rdn1)
                p1 = None
            if p2 is not None:
                stage_b(*p2)
                if p2[1] + p2[2] == HW:
                    finish_channel(p2[0])
                p2 = None
            p2 = nxt2
            p1 = p0
            p0 = nxt

        for k, (it, start, size) in enumerate(steps):
            if start == 0:
                o_t = outp.tile([128, NBLK, 2], f32, tag="o_t")
                o_ts[it] = o_t
            rrw = stage_a1(it, start, size, k)
            advance((it, start, size, rrw))
        advance(None)
        advance(None)
        advance(None)
    nc.finalize()
    return nc


def _host_scalars(x, radius_mean, angle_mean, radius_std, angle_std):
    """Per-core scalar tables [128, CL] + FIX [128, 64*CL].

    partition = b*64 + o.  FIX[8j, 4i*2 + b, for channel slot] = br^2 - lx
    at origin o=(i,j)'s own pixel (h=8i, w=8j -> block 4i, partition 8j).
    """
    inv2rv = 1.0 / (2.0 * (radius_std.astype(np.float64) ** 2 + 0.01))   # [C]
    inv2av = 1.0 / (2.0 * (angle_std.astype(np.float64) ** 2 + 0.0001))  # [C]
    rm = np.abs(radius_mean.astype(np.float64)).reshape(B, C, NO)
    am = angle_mean.astype(np.float64).reshape(B, C, NO)
    xx = np.maximum(x.astype(np.float64).reshape(B, C, NO), 1e-30)
    per_core = []
    for k in range(NCORES):
        cs = np.arange(k * CL, (k + 1) * CL)
        a2 = np.zeros((128, CL)); s2 = np.zeros((128, CL))
        sr = np.zeros((128, CL)); br = np.zeros((128, CL))
        lxv = np.zeros((128, CL)); s2a2 = np.zeros((128, CL))
        fix = np.full((128, CL, NBLK, 2), 1e30)
        for itc, c in enumerate(cs):
            srt = np.sqrt(inv2rv[c])
            for b in range(B):
                p = slice(b * NO, (b + 1) * NO)
                a2r = -am[b, c] / TWO_PI
                a2w = a2r - np.round(a2r)              # wrap to [-0.5, 0.5]
                a2[p, itc] = a2w
                s2v = TWO_PI * np.sqrt(inv2av[c])
                s2[p, itc] = s2v
                sr[p, itc] = srt
                br[p, itc] = -rm[b, c] * srt
                lxv[p, itc] = np.log(xx[b, c])
                s2a2[p, itc] = -s2v * a2w              # sqa bias (-s2*a2)
                for i in range(NH):
                    for j in range(NW):
                        o = i * NW + j
                        brv = np.float32(-rm[b, c, o] * srt)
                        lv = np.float32(np.log(xx[b, c, o]))
                        fix[8 * j, itc, 4 * i, b] = (
                            np.float32(brv * brv) - lv)
        f = lambda a: np.ascontiguousarray(a.astype(np.float32))
        per_core.append(dict(a2=f(a2), s2=f(s2), sr=f(sr), br=f(br),
                             lx=f(lxv), s2a2=f(s2a2),
                             fix=f(fix.reshape(128, CL * NBLK * 2))))
    return per_core


def kernel(x, radius_mean, angle_mean, radius_std, angle_std, _trace=False,
           _tmpdir=None):
    from concourse.bass_utils import run_bass_kernel_spmd

    if "nc" not in _built:
        _built["nc"] = _build_bass()
        _built["fields"] = _host_fields()
    nc = _built["nc"]
    v, fn = _built["fields"]
    fld = np.concatenate([v, fn, -v], axis=1)          # [64, 3*HW]
    vh1 = np.concatenate([v, -v], axis=1)              # [64, 2*HW]
    vh = np.concatenate([vh1, vh1], axis=0).astype(np.float16)
    fld2 = np.concatenate([fld, fld], axis=0)          # [128, 2*HW]
    ident = np.eye(128, dtype=np.float32)
    sc = _host_scalars(x, radius_mean, angle_mean, radius_std, angle_std)
    in_maps = []
    for k in range(NCORES):
        s = sc[k]
        ncb = np.full((128, 1), -1536.0, dtype=np.float32)
        scal = np.concatenate(
            [s["a2"], s["s2"], s["sr"], s["br"], s["lx"], s["s2a2"],
             ncb, s["fix"]], axis=1)
        cst = np.ascontiguousarray(
            np.concatenate([fld2, ident, scal], axis=1))
        s16 = np.concatenate(
            [s["a2"], np.full((128, 1), -1536.0, dtype=np.float32)],
            axis=1).astype(np.float16)
        import ml_dtypes
        lxr = np.empty((1, 128 + 512 * CL), dtype=np.float32)
        lxr[0, :128] = 1.0
        for itc in range(CL):
            nlx = -s["lx"][:, itc]                 # [128] = per (b,o)
            lxr[0, 128 + itc * 512 : 128 + (itc + 1) * 512] = np.tile(nlx, 4)
        lxr = lxr.astype(ml_dtypes.bfloat16)
        in_maps.append({"cst": cst, "vh": vh,
                        "s16": np.ascontiguousarray(s16),
                        "lxr": np.ascontiguousarray(lxr)})
    res = run_bass_kernel_spmd(nc, in_maps, core_ids=list(range(NCORES)),
                               trace=_trace, tmpdir=_tmpdir)
    if _trace:
        return res
    out = np.empty((B, C, H, W), dtype=np.float32)
    for k in range(NCORES):
        r = res.results[k]["out"]          # [CL, 128, NBLK, 2]
        # value at [it, p, blk, b] is pixel blk*128+p of batch b, channel it
        r = r.transpose(3, 0, 2, 1).reshape(B, CL, H, W)
        out[:, k * CL : (k + 1) * CL] = r
    return out
